# revision 30
# baseline (speedup 1.0000x reference)
"""Trainium2 Bass kernel: multi-head attention (B=4, T=2048, D=768, H=12).

Sharding: 8 cores = 4 batches x 2 head-groups (6 heads each).
Each core computes QKV projection (its heads), attention, and a partial
output projection (contraction over its 384 of 768 w_out rows).
Host unshard: out[b] = f32(partial[2b]) + f32(partial[2b+1]) + b_out.

Fully pipelined single-stream schedule. The ScalarE exp stream (192
tiles of [128,1024], ~213us busy) is the kernel floor; everything else
is arranged around keeping it gap-free:

  - bf16 inputs, DMA'd in consumption order. x^T is laid out in DRAM by
    token-chunk (all six D-tiles of a 512-token chunk contiguous), so
    the first QK projection starts after ~1.6MB, not 4.2MB. q-chunk 0
    (tokens 0:1024) is processed first for the same reason.
  - phase 2 is one flat stream over (unit, kb) steps; S/exp of step g
    and the PV of step g-LEAD are emitted together, crossing unit
    boundaries so the PE never drains PV work while ACT waits for S.
  - all remaining projection and out-projection work is emitted as
    fillers inside designated units (after that step's S matmuls), at a
    density the PE can absorb without starving ACT.
  - softmax denominators ride the PV matmul as a 65th lhsT column of
    ones; the normalize chain per unit is: DVE evict of au, DMA-reshape
    of the denominator row to [128, qc/128], DVE reciprocal (all lanes),
    DMA back into a zero-padded r_pad row, rank-1 PE matmul broadcast
    into 1-bank pj PSUM slots, DVE multiply into AN. The chain is
    deferred two PV-steps so none of it gates the S stream.
  - dummy matmuls warm the PE HAM clock gate during the DMA head and
    the final reciprocal round-trip.
  - PSUM budget (8 banks): st 2x[128,1024] (4) + au [65,1024] (2) +
    shared 1-bank pj pool x2 (2) used by V-proj/QK-proj/out-proj
    fillers, warmup dummies, and the normalize broadcasts.

This walrus build encodes at most one sync wait per instruction; Tile
emits several. _split_multi_waits() hoists extras onto same-engine nops.
"""

import numpy as np
import ml_dtypes

import concourse.bass as bass
import concourse.mybir as mybir
from concourse.tile import TileContext
from concourse.bass_utils import run_bass_kernel_spmd

# problem constants (fixed by the graded nn.Module)
B, T, D = 4, 2048, 768
H, HD = 12, 64
NCORES = 8
HL = H // 2            # heads per core
NPAIR = HL // 2        # head pairs per core

F32 = mybir.dt.float32
BF16 = mybir.dt.bfloat16
BF = ml_dtypes.bfloat16
COPY = mybir.ActivationFunctionType.Copy
IDENT = mybir.ActivationFunctionType.Identity


def _split_multi_waits(nc):
    """Walrus here encodes only one sync wait per instruction. Move extra
    waits onto same-engine nops placed immediately before the instruction."""
    n = 0
    for f in nc.m.functions:
        for bb in f.blocks:
            new = []
            for inst in bb.instructions:
                si = inst.sync_info
                if si is not None and si.on_wait and len(si.on_wait) > 1:
                    extra = list(si.on_wait[:-1])
                    keep = si.on_wait[-1]
                    del si.on_wait[:]
                    si.on_wait.append(keep)
                    for w in extra:
                        nop = mybir.InstNoOp(name=f"I-wsplit-{n}", ins=[], outs=[])
                        n += 1
                        nop.engine = inst.engine
                        nop.sync_info = mybir.SyncInfo(on_wait=[w], on_update=[])
                        new.append(nop)
                new.append(inst)
            bb.instructions[:] = new
    return n


def build_nc(t=T, qc=1024, nch=512):
    tokt = t // 128            # token tiles (16)
    dk = D // 128              # contraction tiles over D (6)
    nmt = 2 * HL * HD // 128   # QK projection M-tiles (6)
    ncc = t // nch             # token chunks (4)
    EXP = mybir.ActivationFunctionType.Exp

    nc = bass.Bass("TRN2", target_bir_lowering=False, debug=False)

    # x^T by token-chunk: col = c*(dk*nch) + k*nch + tok
    xt_d = nc.dram_tensor("xt", [128, ncc * dk * nch], BF16, kind="ExternalInput")
    # weights k-banded: col = k*width + in-tile col
    wqk0_d = nc.dram_tensor("wqk0", [128, dk * 256], BF16, kind="ExternalInput")
    wqkr_d = nc.dram_tensor("wqkr", [128, dk * 512], BF16, kind="ExternalInput")
    wv_d = nc.dram_tensor("wv", [128, dk * HL * HD], BF16, kind="ExternalInput")
    wvb_d = nc.dram_tensor("wvb", [1, HL * HD], BF16, kind="ExternalInput")
    bqk_d = nc.dram_tensor("bqk", [128, nmt], F32, kind="ExternalInput")
    wo_d = nc.dram_tensor("wo", [HL * HD, D], BF16, kind="ExternalInput")
    out_d = nc.dram_tensor("out", [t, D], BF16, kind="ExternalOutput")

    def MM(out, lhsT, rhs, start, stop):
        nc.tensor.matmul(out, lhsT, rhs, start=start, stop=stop)

    with TileContext(nc) as tc:
        lp = nc.allow_low_precision(reason="bf16 compute pipeline")
        lp.__enter__()
        with tc.tile_pool(name="persist", bufs=1) as pp:
            ones128 = pp.tile([128, 128], BF16, name="ones128")
            r_pads = [pp.tile([128, qc], BF16, name=f"r_pad{i}") for i in range(2)]
            QT = [pp.tile([128, t], BF16, name=f"qt{p}") for p in range(NPAIR)]
            KT = [pp.tile([128, t], BF16, name=f"kt{h}") for h in range(HL)]
            V6 = [pp.tile([128, HL * (HD + 1)], BF16, name=f"v6_{c}") for c in range(tokt)]
            AN = [pp.tile([128, t], BF16, name=f"an{p}") for p in range(NPAIR)]
            WO = [pp.tile([128, D], BF16, name=f"wop{p}") for p in range(NPAIR)]
            bqk_t = pp.tile([128, nmt], F32, name="bqk_t")
            xt_all = pp.tile([128, dk * t], BF16, name="xt_all")
            wqk0_t = pp.tile([128, dk * 256], BF16, name="wqk0_t")
            wqkr_t = pp.tile([128, dk * 512], BF16, name="wqkr_t")
            wv_t = pp.tile([128, dk * HL * HD], BF16, name="wv_t")
            wvb = pp.tile([128, HL * HD], BF16, name="wvb")

            # constants first (wvb memset must precede its row DMA)
            nc.vector.memset(ones128[:], 0.0)
            nc.vector.memset(ones128[0:1, :], 1.0)
            for i in range(2):
                nc.vector.memset(r_pads[i][:], 0.0)
            nc.vector.memset(wvb[:], 0.0)
            for h in range(HL):
                if h % 2 == 0:
                    nc.vector.memset(KT[h][64:128, :], 0.0)
                else:
                    nc.vector.memset(KT[h][0:64, :], 0.0)

            # DMAs in consumption order; x^T chunks land one by one
            xt3 = xt_all[:].rearrange("p (k tt) -> p k tt", tt=t)
            nc.sync.dma_start(out=bqk_t[:], in_=bqk_d[:, :])
            for c in (0, 1):
                nc.sync.dma_start(
                    out=xt3[:, :, c * nch:(c + 1) * nch],
                    in_=xt_d[:, c * dk * nch:(c + 1) * dk * nch])
            nc.sync.dma_start(out=wqk0_t[:], in_=wqk0_d[:, :])
            nc.sync.dma_start(out=wv_t[:], in_=wv_d[:, :])
            nc.sync.dma_start(out=wvb[0:1, :], in_=wvb_d[:, :])
            for c in (2, 3):
                nc.sync.dma_start(
                    out=xt3[:, :, c * nch:(c + 1) * nch],
                    in_=xt_d[:, c * dk * nch:(c + 1) * dk * nch])
            nc.sync.dma_start(out=wqkr_t[:], in_=wqkr_d[:, :])
            for p in range(NPAIR):
                nc.sync.dma_start(out=WO[p][:], in_=wo_d[p * 128:(p + 1) * 128, :])

            with (
                tc.tile_pool(name="psum_s", bufs=2, space="PSUM") as s_pool,
                tc.tile_pool(name="psum_u", bufs=1, space="PSUM") as u_pool,
                tc.tile_pool(name="psum_p", bufs=2, space="PSUM") as p_pool,
                tc.tile_pool(name="sb_wk", bufs=2) as wk,
                tc.tile_pool(name="sb_pt", bufs=4) as ptp,
                tc.tile_pool(name="sb_so", bufs=3) as sop,
            ):
                def qk_mtile_chunk(m, c, on_act=False):
                    """One [128,512] token-chunk of QK-projection M-tile m, with
                    bias eviction into QT / zero-padded KT."""
                    csl = slice(c * nch, (c + 1) * nch)
                    ps = p_pool.tile([128, nch], F32, tag="pj", bufs=2, name="psqk")
                    for k in range(dk):
                        if m < 2:
                            w = wqk0_t[:, k * 256 + m * 128:k * 256 + (m + 1) * 128]
                        else:
                            w = wqkr_t[:, k * 512 + (m - 2) * 128:k * 512 + (m - 1) * 128]
                        MM(ps[:], w, xt_all[:, k * t + c * nch:k * t + (c + 1) * nch],
                           start=(k == 0), stop=(k == dk - 1))
                    if m % 2 == 0:
                        if on_act:
                            nc.scalar.activation(QT[m // 2][:, csl], ps[:], IDENT,
                                                 bias=bqk_t[:, m:m + 1])
                        else:
                            nc.vector.tensor_scalar_add(
                                QT[m // 2][:, csl], ps[:], bqk_t[:, m:m + 1])
                    else:
                        pr = m // 2
                        if on_act:
                            nc.scalar.activation(KT[2 * pr][0:64, csl], ps[0:64, :],
                                                 IDENT, bias=bqk_t[0:64, m:m + 1])
                            nc.scalar.activation(KT[2 * pr + 1][64:128, csl],
                                                 ps[64:128, :], IDENT,
                                                 bias=bqk_t[64:128, m:m + 1])
                        else:
                            nc.vector.tensor_scalar_add(
                                KT[2 * pr][0:64, csl], ps[0:64, :], bqk_t[0:64, m:m + 1])
                            nc.vector.tensor_scalar_add(
                                KT[2 * pr + 1][64:128, csl], ps[64:128, :],
                                bqk_t[64:128, m:m + 1])

                def v_ctile(c, on_act=False):
                    """V projection for token tile c -> V6[c] (with ones col)."""
                    tsl = slice(c * 128, (c + 1) * 128)
                    psv = p_pool.tile([128, HL * HD], F32, tag="pj", bufs=2, name="psv")
                    for k in range(dk):
                        MM(psv[:], xt_all[:, k * t + c * 128:k * t + (c + 1) * 128],
                           wv_t[:, k * HL * HD:(k + 1) * HL * HD],
                           start=(k == 0), stop=False)
                    MM(psv[:], ones128[:], wvb[:], start=False, stop=True)
                    v3 = V6[c][:].rearrange("p (h c2) -> p h c2", c2=HD + 1)
                    nc.vector.memset(v3[:, :, HD:HD + 1], 1.0)
                    src = psv[:].rearrange("p (h c2) -> p h c2", c2=HD)
                    if on_act:
                        nc.scalar.activation(v3[:, :, 0:HD], src, COPY)
                    else:
                        nc.vector.tensor_copy(v3[:, :, 0:HD], src)

                def oproj_tile(c):
                    """Output projection for token tile c (two 384-col halves,
                    one eviction tile, one output DMA)."""
                    tsl = slice(c * 128, (c + 1) * 128)
                    so = sop.tile([128, D], BF16, tag="so", bufs=3, name="so")
                    for half in range(2):
                        nsl = slice(half * 384, (half + 1) * 384)
                        ps = p_pool.tile([128, 384], F32, tag="pj", bufs=2, name="pso")
                        for p in range(NPAIR):
                            MM(ps[:], AN[p][:, tsl], WO[p][:, nsl],
                               start=(p == 0), stop=(p == NPAIR - 1))
                        nc.vector.tensor_copy(so[:, nsl], ps[:])
                    eng = nc.sync if c % 2 == 0 else nc.scalar
                    eng.dma_start(out=out_d[tsl, :], in_=so[:])

                # filler schedule (q0-chunk first): unit -> [(kb, closure)...]
                # emitted AFTER that step's S matmuls
                def QKf(m, c):
                    return lambda: qk_mtile_chunk(m, c)

                fillers = {u: [] for u in range(2 * HL)}
                fillers[0] = sorted(
                    [(kb, (lambda cc=kb + 3: v_ctile(cc))) for kb in range(tokt - 3)]
                    + [(1, QKf(1, 2)), (2, QKf(1, 3))],
                    key=lambda kv: kv[0])
                fillers[1] = [(0, QKf(2, 0)), (2, QKf(2, 1)), (4, QKf(3, 0)),
                              (6, QKf(3, 1)), (8, QKf(3, 2)), (10, QKf(3, 3))]
                fillers[2] = [(0, QKf(4, 0)), (2, QKf(4, 1)), (4, QKf(5, 0)),
                              (6, QKf(5, 1)), (8, QKf(5, 2)), (10, QKf(5, 3))]
                fillers[3] = [(4, QKf(0, 2)), (10, QKf(0, 3))]
                fillers[4] = [(4, QKf(2, 2)), (10, QKf(2, 3))]
                fillers[5] = [(4, QKf(4, 2)), (10, QKf(4, 3))]
                for i, c in enumerate(range(0, 8)):
                    fillers[6 + i // 2].append(
                        (5 + 8 * (i % 2), (lambda cc=c: oproj_tile(cc))))

                # dummy matmuls with no DMA deps keep the PE busy during the
                # input-DMA head so the HAM clock gate reaches 2.4GHz early
                wdum = p_pool.tile([128, nch], F32, tag="pj", bufs=2, name="wdum")
                for _ in range(28):
                    MM(wdum[:], ones128[:], r_pads[0][:, 0:nch], start=True, stop=True)

                # QK projection for pair 0, q-chunk 0 (gates the first S) and
                # the first V tiles; evictions ride the still-idle ScalarE
                for m in (0, 1):
                    for c in (0, 1):
                        qk_mtile_chunk(m, c)
                for c in range(3):
                    v_ctile(c)

                # ---- phase 2: one flat stream over (unit, kb) ----
                units = [(p, j, q) for q in (0, 1)
                         for p in range(NPAIR) for j in range(2)]
                LEAD = 3
                GS = len(units) * tokt
                au_t = {}
                pt_t = {}
                fidx = {u: 0 for u in range(len(units))}
                pend = []

                def chain_head(item):
                    pc, pj_, pq0, pauSB, prpad = item
                    dT = wk.tile([128, qc // 128], F32, tag="dt", bufs=2, name="dt")
                    nc.sync.dma_start(out=dT[:], in_=pauSB[64:65, :])
                    rT = wk.tile([128, qc // 128], BF16, tag="rt", bufs=2, name="rt")
                    nc.vector.reciprocal(rT[:], dT[:])
                    nc.sync.dma_start(out=prpad[0:1, :], in_=rT[:])

                def chain_tail(item):
                    pc, pj_, pq0, pauSB, prpad = item
                    for c in range(qc // nch):
                        csl = slice(c * nch, (c + 1) * nch)
                        R = p_pool.tile([64, nch], F32, tag="pj", bufs=2, name="Rb")
                        MM(R[:], ones128[:, 0:64], prpad[:, csl],
                           start=True, stop=True)
                        nc.vector.tensor_mul(
                            AN[pc][pj_ * 64:(pj_ + 1) * 64,
                                   pq0 + c * nch:pq0 + (c + 1) * nch],
                            pauSB[0:64, csl], R[:])

                def emit_sa(g):
                    u, kb = divmod(g, tokt)
                    p, j, q = units[u]
                    h = 2 * p + j
                    q0 = q * qc
                    st = s_pool.tile([128, qc], F32, tag="st", bufs=2, name="st")
                    for c in range(qc // nch):
                        MM(st[:, c * nch:(c + 1) * nch],
                           KT[h][:, kb * 128:(kb + 1) * 128],
                           QT[p][:, q0 + c * nch:q0 + (c + 1) * nch],
                           start=True, stop=True)
                    pt = ptp.tile([128, qc], BF16, tag="pt", bufs=LEAD + 1, name="pt")
                    nc.scalar.activation(pt[:], st[:], EXP, scale=0.125)
                    pt_t[g] = pt
                    flist = fillers[u]
                    while fidx[u] < len(flist) and flist[fidx[u]][0] <= kb:
                        flist[fidx[u]][1]()
                        fidx[u] += 1

                def emit_pv(g):
                    u, kb = divmod(g, tokt)
                    p, j, q = units[u]
                    h = 2 * p + j
                    if kb == 0:
                        au_t[u] = u_pool.tile([65, qc], F32, tag="au", bufs=1, name="au")
                    au = au_t[u]
                    pt = pt_t.pop(g)
                    for c in range(qc // nch):
                        MM(au[:, c * nch:(c + 1) * nch],
                           V6[kb][:, h * (HD + 1):(h + 1) * (HD + 1)],
                           pt[:, c * nch:(c + 1) * nch],
                           start=(kb == 0), stop=(kb == tokt - 1))
                    if kb == tokt - 1:
                        auSB = wk.tile([65, qc], F32, tag="ausb", bufs=3, name="ausb")
                        nc.vector.tensor_copy(auSB[:], au[:])
                        del au_t[u]
                        pend.append([2, (p, j, q * qc, auSB, r_pads[u % 2])])

                def tick_chains():
                    for it in pend:
                        it[0] -= 1
                    while pend and pend[0][0] <= 0:
                        it = pend.pop(0)[1]
                        chain_head(it)
                        chain_tail(it)

                for g in range(GS):
                    emit_sa(g)
                    if g >= LEAD:
                        emit_pv(g - LEAD)
                        tick_chains()
                for g in range(GS - LEAD, GS):
                    emit_pv(g)
                # final unit's chain: start the reciprocal round-trip, keep the
                # PE clock warm with dummies while it lands, then finish
                last = pend.pop(0)[1]
                chain_head(last)
                for _ in range(36):
                    MM(wdum[:], ones128[:], r_pads[0][:, 0:nch], start=True, stop=True)
                chain_tail(last)
                # trailing out-proj for the last q-chunk (tokens 1024:2048)
                for c in range(8, 16):
                    oproj_tile(c)
        lp.__exit__(None, None, None)

    return nc


def shard_inputs(x, w_qkv, b_qkv, w_out, b_out, t=T):
    """Build the 8 per-core input maps. Core = (batch, head-group)."""
    dk = D // 128
    nch = 512
    ncc = t // nch
    in_maps = []
    for core in range(NCORES):
        b, g = divmod(core, 2)
        hbase = HL * g * HD
        # q cols then k cols, pair-interleaved: M-tile 2p = q of heads (2p,2p+1),
        # M-tile 2p+1 = k of the same heads.
        wqk = np.empty((D, 2 * HL * HD), dtype=np.float32)
        bqk = np.empty((2 * HL * HD,), dtype=np.float32)
        for p in range(NPAIR):
            qcols = slice(0 * D + hbase + p * 128, 0 * D + hbase + (p + 1) * 128)
            kcols = slice(1 * D + hbase + p * 128, 1 * D + hbase + (p + 1) * 128)
            wqk[:, (2 * p) * 128:(2 * p + 1) * 128] = w_qkv[:, qcols]
            wqk[:, (2 * p + 1) * 128:(2 * p + 2) * 128] = w_qkv[:, kcols]
            bqk[(2 * p) * 128:(2 * p + 1) * 128] = b_qkv[qcols]
            bqk[(2 * p + 1) * 128:(2 * p + 2) * 128] = b_qkv[kcols]
        nmt = 2 * HL * HD // 128
        bqk_col = np.ascontiguousarray(bqk.reshape(nmt, 128).T)  # [128, nmt]

        vcols = slice(2 * D + hbase, 2 * D + hbase + HL * HD)
        wv = np.ascontiguousarray(w_qkv[:, vcols])               # [768, 384]
        wvb_row = b_qkv[vcols].reshape(1, HL * HD)

        xT = np.ascontiguousarray(x[b, :t].T)                    # [768, 2048]
        # xt: [k, p, c, tok] -> [p, c, k, tok]
        xtA = xT.reshape(dk, 128, ncc, nch).transpose(1, 2, 0, 3).reshape(
            128, ncc * dk * nch)
        # weights k-banded: [k, p, col] -> [p, k, col]
        wqk0A = wqk[:, 0:256].reshape(dk, 128, 256).transpose(1, 0, 2).reshape(
            128, dk * 256)
        wqkrA = wqk[:, 256:768].reshape(dk, 128, 512).transpose(1, 0, 2).reshape(
            128, dk * 512)
        wvA = wv.reshape(dk, 128, HL * HD).transpose(1, 0, 2).reshape(
            128, dk * HL * HD)

        wo = np.ascontiguousarray(w_out[hbase:hbase + HL * HD, :])

        in_maps.append(
            {
                "xt": np.ascontiguousarray(xtA).astype(BF),
                "wqk0": np.ascontiguousarray(wqk0A).astype(BF),
                "wqkr": np.ascontiguousarray(wqkrA).astype(BF),
                "bqk": bqk_col.astype(np.float32),
                "wv": np.ascontiguousarray(wvA).astype(BF),
                "wvb": wvb_row.astype(BF),
                "wo": wo.astype(BF),
            }
        )
    return in_maps


def kernel(x, w_qkv, b_qkv, w_out, b_out):
    x = np.asarray(x, dtype=np.float32)
    w_qkv = np.asarray(w_qkv, dtype=np.float32)
    b_qkv = np.asarray(b_qkv, dtype=np.float32)
    w_out = np.asarray(w_out, dtype=np.float32)
    b_out = np.asarray(b_out, dtype=np.float32)

    nc = build_nc()
    _split_multi_waits(nc)
    in_maps = shard_inputs(x, w_qkv, b_qkv, w_out, b_out)
    res = run_bass_kernel_spmd(nc, in_maps, list(range(NCORES)))
    parts = [np.asarray(res.results[i]["out"]).astype(np.float32) for i in range(NCORES)]
    out = np.stack([parts[2 * b] + parts[2 * b + 1] for b in range(B)], axis=0)
    out += b_out[None, None, :]
    return out.astype(np.float32)


# revision 31
# speedup vs baseline: 1.0123x; 1.0123x over previous
"""Trainium2 Bass kernel: multi-head attention (B=4, T=2048, D=768, H=12).

Sharding: 8 cores = 4 batches x 2 head-groups (6 heads each).
Each core computes QKV projection (its heads), attention, and a partial
output projection (contraction over its 384 of 768 w_out rows).
Host unshard: out[b] = f32(partial[2b]) + f32(partial[2b+1]) + b_out.

Fully pipelined single-stream schedule. The ScalarE exp stream (192
tiles of [128,1024], ~213us busy) is the kernel floor; everything else
is arranged around keeping it gap-free:

  - bf16 inputs, DMA'd in consumption order. x^T is laid out in DRAM by
    token-chunk (all six D-tiles of a 512-token chunk contiguous), so
    the first QK projection starts after ~1.6MB, not 4.2MB. q-chunk 0
    (tokens 0:1024) is processed first for the same reason.
  - phase 2 is one flat stream over (unit, kb) steps; S/exp of step g
    and the PV of step g-LEAD are emitted together, crossing unit
    boundaries so the PE never drains PV work while ACT waits for S.
  - all remaining projection and out-projection work is emitted as
    fillers inside designated units (after that step's S matmuls), at a
    density the PE can absorb without starving ACT.
  - softmax denominators ride the PV matmul as a 65th lhsT column of
    ones; the normalize chain per unit is: DVE evict of au, DMA-reshape
    of the denominator row to [128, qc/128], DVE reciprocal (all lanes),
    DMA back into a zero-padded r_pad row, rank-1 PE matmul broadcast
    into 1-bank pj PSUM slots, DVE multiply into AN. The chain is
    deferred two PV-steps so none of it gates the S stream.
  - dummy matmuls warm the PE HAM clock gate during the DMA head and
    the final reciprocal round-trip.
  - PSUM budget (8 banks): st 2x[128,1024] (4) + au [65,1024] (2) +
    shared 1-bank pj pool x2 (2) used by V-proj/QK-proj/out-proj
    fillers, warmup dummies, and the normalize broadcasts.

This walrus build encodes at most one sync wait per instruction; Tile
emits several. _split_multi_waits() hoists extras onto same-engine nops.
"""

import numpy as np
import ml_dtypes

import concourse.bass as bass
import concourse.mybir as mybir
from concourse.tile import TileContext
from concourse.bass_utils import run_bass_kernel_spmd

# problem constants (fixed by the graded nn.Module)
B, T, D = 4, 2048, 768
H, HD = 12, 64
NCORES = 8
HL = H // 2            # heads per core
NPAIR = HL // 2        # head pairs per core

F32 = mybir.dt.float32
BF16 = mybir.dt.bfloat16
BF = ml_dtypes.bfloat16
COPY = mybir.ActivationFunctionType.Copy
IDENT = mybir.ActivationFunctionType.Identity


def _split_multi_waits(nc):
    """Walrus here encodes only one sync wait per instruction. Move extra
    waits onto same-engine nops placed immediately before the instruction."""
    n = 0
    for f in nc.m.functions:
        for bb in f.blocks:
            new = []
            for inst in bb.instructions:
                si = inst.sync_info
                if si is not None and si.on_wait and len(si.on_wait) > 1:
                    extra = list(si.on_wait[:-1])
                    keep = si.on_wait[-1]
                    del si.on_wait[:]
                    si.on_wait.append(keep)
                    for w in extra:
                        nop = mybir.InstNoOp(name=f"I-wsplit-{n}", ins=[], outs=[])
                        n += 1
                        nop.engine = inst.engine
                        nop.sync_info = mybir.SyncInfo(on_wait=[w], on_update=[])
                        new.append(nop)
                new.append(inst)
            bb.instructions[:] = new
    return n


def build_nc(t=T, qc=1024, nch=512):
    tokt = t // 128            # token tiles (16)
    dk = D // 128              # contraction tiles over D (6)
    nmt = 2 * HL * HD // 128   # QK projection M-tiles (6)
    ncc = t // nch             # token chunks (4)
    EXP = mybir.ActivationFunctionType.Exp

    nc = bass.Bass("TRN2", target_bir_lowering=False, debug=False)

    # x^T by token-chunk: col = c*(dk*nch) + k*nch + tok
    xt_d = nc.dram_tensor("xt", [128, ncc * dk * nch], BF16, kind="ExternalInput")
    # weights k-banded: col = k*width + in-tile col
    wqk0_d = nc.dram_tensor("wqk0", [128, dk * 256], BF16, kind="ExternalInput")
    wqkr_d = nc.dram_tensor("wqkr", [128, dk * 512], BF16, kind="ExternalInput")
    wv_d = nc.dram_tensor("wv", [128, dk * HL * HD], BF16, kind="ExternalInput")
    wvb_d = nc.dram_tensor("wvb", [1, HL * HD], BF16, kind="ExternalInput")
    bqk_d = nc.dram_tensor("bqk", [128, nmt], F32, kind="ExternalInput")
    wo_d = nc.dram_tensor("wo", [HL * HD, D], BF16, kind="ExternalInput")
    out_d = nc.dram_tensor("out", [t, D], BF16, kind="ExternalOutput")

    def MM(out, lhsT, rhs, start, stop):
        nc.tensor.matmul(out, lhsT, rhs, start=start, stop=stop)

    with TileContext(nc) as tc:
        lp = nc.allow_low_precision(reason="bf16 compute pipeline")
        lp.__enter__()
        with tc.tile_pool(name="persist", bufs=1) as pp:
            ones128 = pp.tile([128, 128], BF16, name="ones128")
            r_pads = [pp.tile([128, qc], BF16, name=f"r_pad{i}") for i in range(2)]
            QT = [pp.tile([128, t], BF16, name=f"qt{p}") for p in range(NPAIR)]
            KT = [pp.tile([128, t], BF16, name=f"kt{h}") for h in range(HL)]
            V6 = [pp.tile([128, HL * (HD + 1)], BF16, name=f"v6_{c}") for c in range(tokt)]
            AN = [pp.tile([128, t], BF16, name=f"an{p}") for p in range(NPAIR)]
            WO = [pp.tile([128, D], BF16, name=f"wop{p}") for p in range(NPAIR)]
            bqk_t = pp.tile([128, nmt], F32, name="bqk_t")
            xt_all = pp.tile([128, dk * t], BF16, name="xt_all")
            wqk0_t = pp.tile([128, dk * 256], BF16, name="wqk0_t")
            wqkr_t = pp.tile([128, dk * 512], BF16, name="wqkr_t")
            wv_t = pp.tile([128, dk * HL * HD], BF16, name="wv_t")
            wvb = pp.tile([128, HL * HD], BF16, name="wvb")

            # constants first (wvb memset must precede its row DMA)
            nc.vector.memset(ones128[:], 0.0)
            nc.vector.memset(ones128[0:1, :], 1.0)
            for i in range(2):
                nc.vector.memset(r_pads[i][:], 0.0)
            nc.vector.memset(wvb[:], 0.0)
            for h in range(HL):
                if h % 2 == 0:
                    nc.vector.memset(KT[h][64:128, :], 0.0)
                else:
                    nc.vector.memset(KT[h][0:64, :], 0.0)

            # DMAs in consumption order; x^T chunks land one by one
            xt3 = xt_all[:].rearrange("p (k tt) -> p k tt", tt=t)
            nc.sync.dma_start(out=bqk_t[:], in_=bqk_d[:, :])
            for c in (0, 1):
                nc.sync.dma_start(
                    out=xt3[:, :, c * nch:(c + 1) * nch],
                    in_=xt_d[:, c * dk * nch:(c + 1) * dk * nch])
            nc.sync.dma_start(out=wqk0_t[:], in_=wqk0_d[:, :])
            nc.sync.dma_start(out=wv_t[:], in_=wv_d[:, :])
            nc.sync.dma_start(out=wvb[0:1, :], in_=wvb_d[:, :])
            for c in (2, 3):
                nc.sync.dma_start(
                    out=xt3[:, :, c * nch:(c + 1) * nch],
                    in_=xt_d[:, c * dk * nch:(c + 1) * dk * nch])
            nc.sync.dma_start(out=wqkr_t[:], in_=wqkr_d[:, :])
            for p in range(NPAIR):
                nc.sync.dma_start(out=WO[p][:], in_=wo_d[p * 128:(p + 1) * 128, :])

            with (
                tc.tile_pool(name="psum_s", bufs=2, space="PSUM") as s_pool,
                tc.tile_pool(name="psum_u", bufs=1, space="PSUM") as u_pool,
                tc.tile_pool(name="psum_p", bufs=2, space="PSUM") as p_pool,
                tc.tile_pool(name="sb_wk", bufs=2) as wk,
                tc.tile_pool(name="sb_pt", bufs=4) as ptp,
                tc.tile_pool(name="sb_so", bufs=3) as sop,
            ):
                def qk_mtile_chunk(m, c, on_act=False):
                    """One [128,512] token-chunk of QK-projection M-tile m, with
                    bias eviction into QT / zero-padded KT."""
                    csl = slice(c * nch, (c + 1) * nch)
                    ps = p_pool.tile([128, nch], F32, tag="pj", bufs=2, name="psqk")
                    for k in range(dk):
                        if m < 2:
                            w = wqk0_t[:, k * 256 + m * 128:k * 256 + (m + 1) * 128]
                        else:
                            w = wqkr_t[:, k * 512 + (m - 2) * 128:k * 512 + (m - 1) * 128]
                        MM(ps[:], w, xt_all[:, k * t + c * nch:k * t + (c + 1) * nch],
                           start=(k == 0), stop=(k == dk - 1))
                    if m % 2 == 0:
                        if on_act:
                            nc.scalar.activation(QT[m // 2][:, csl], ps[:], IDENT,
                                                 bias=bqk_t[:, m:m + 1])
                        else:
                            nc.vector.tensor_scalar_add(
                                QT[m // 2][:, csl], ps[:], bqk_t[:, m:m + 1])
                    else:
                        pr = m // 2
                        if on_act:
                            nc.scalar.activation(KT[2 * pr][0:64, csl], ps[0:64, :],
                                                 IDENT, bias=bqk_t[0:64, m:m + 1])
                            nc.scalar.activation(KT[2 * pr + 1][64:128, csl],
                                                 ps[64:128, :], IDENT,
                                                 bias=bqk_t[64:128, m:m + 1])
                        else:
                            nc.vector.tensor_scalar_add(
                                KT[2 * pr][0:64, csl], ps[0:64, :], bqk_t[0:64, m:m + 1])
                            nc.vector.tensor_scalar_add(
                                KT[2 * pr + 1][64:128, csl], ps[64:128, :],
                                bqk_t[64:128, m:m + 1])

                def v_ctile(c, on_act=False):
                    """V projection for token tile c -> V6[c] (with ones col)."""
                    tsl = slice(c * 128, (c + 1) * 128)
                    psv = p_pool.tile([128, HL * HD], F32, tag="pj", bufs=2, name="psv")
                    for k in range(dk):
                        MM(psv[:], xt_all[:, k * t + c * 128:k * t + (c + 1) * 128],
                           wv_t[:, k * HL * HD:(k + 1) * HL * HD],
                           start=(k == 0), stop=False)
                    MM(psv[:], ones128[:], wvb[:], start=False, stop=True)
                    v3 = V6[c][:].rearrange("p (h c2) -> p h c2", c2=HD + 1)
                    nc.vector.memset(v3[:, :, HD:HD + 1], 1.0)
                    src = psv[:].rearrange("p (h c2) -> p h c2", c2=HD)
                    if on_act:
                        nc.scalar.activation(v3[:, :, 0:HD], src, COPY)
                    else:
                        nc.vector.tensor_copy(v3[:, :, 0:HD], src)

                def oproj_tile(c):
                    """Output projection for token tile c (two 384-col halves,
                    one eviction tile, one output DMA)."""
                    tsl = slice(c * 128, (c + 1) * 128)
                    so = sop.tile([128, D], BF16, tag="so", bufs=3, name="so")
                    for half in range(2):
                        nsl = slice(half * 384, (half + 1) * 384)
                        ps = p_pool.tile([128, 384], F32, tag="pj", bufs=2, name="pso")
                        for p in range(NPAIR):
                            MM(ps[:], AN[p][:, tsl], WO[p][:, nsl],
                               start=(p == 0), stop=(p == NPAIR - 1))
                        nc.vector.tensor_copy(so[:, nsl], ps[:])
                    eng = nc.scalar if c >= 8 and c % 2 == 1 else nc.sync
                    eng.dma_start(out=out_d[tsl, :], in_=so[:])

                # filler schedule (q0-chunk first): unit -> [(kb, closure)...]
                # emitted AFTER that step's S matmuls
                def QKf(m, c):
                    return lambda: qk_mtile_chunk(m, c)

                fillers = {u: [] for u in range(2 * HL)}
                fillers[0] = sorted(
                    [(kb, (lambda cc=kb + 3: v_ctile(cc))) for kb in range(tokt - 3)]
                    + [(1, QKf(1, 2)), (2, QKf(1, 3))],
                    key=lambda kv: kv[0])
                fillers[1] = [(0, QKf(2, 0)), (2, QKf(2, 1)), (4, QKf(3, 0)),
                              (6, QKf(3, 1)), (8, QKf(3, 2)), (10, QKf(3, 3))]
                fillers[2] = [(0, QKf(4, 0)), (2, QKf(4, 1)), (4, QKf(5, 0)),
                              (6, QKf(5, 1)), (8, QKf(5, 2)), (10, QKf(5, 3))]
                fillers[3] = [(4, QKf(0, 2)), (10, QKf(0, 3))]
                fillers[4] = [(4, QKf(2, 2)), (10, QKf(2, 3))]
                fillers[5] = [(4, QKf(4, 2)), (10, QKf(4, 3))]
                for i, c in enumerate(range(0, 8)):
                    fillers[6 + i // 2].append(
                        (5 + 8 * (i % 2), (lambda cc=c: oproj_tile(cc))))

                # dummy matmuls with no DMA deps keep the PE busy during the
                # input-DMA head so the HAM clock gate reaches 2.4GHz early
                wdum = p_pool.tile([128, nch], F32, tag="pj", bufs=2, name="wdum")
                for _ in range(28):
                    MM(wdum[:], ones128[:], r_pads[0][:, 0:nch], start=True, stop=True)

                # QK projection for pair 0, q-chunk 0 (gates the first S) and
                # the first V tiles; evictions ride the still-idle ScalarE
                for m in (0, 1):
                    for c in (0, 1):
                        qk_mtile_chunk(m, c)
                for c in range(3):
                    v_ctile(c)

                # ---- phase 2: one flat stream over (unit, kb) ----
                units = [(p, j, q) for q in (0, 1)
                         for p in range(NPAIR) for j in range(2)]
                LEAD = 3
                GS = len(units) * tokt
                au_t = {}
                pt_t = {}
                fidx = {u: 0 for u in range(len(units))}
                pend = []

                def chain_head(item):
                    pc, pj_, pq0, pauSB, prpad = item
                    dT = wk.tile([128, qc // 128], F32, tag="dt", bufs=2, name="dt")
                    nc.sync.dma_start(out=dT[:], in_=pauSB[64:65, :])
                    rT = wk.tile([128, qc // 128], BF16, tag="rt", bufs=2, name="rt")
                    nc.vector.reciprocal(rT[:], dT[:])
                    nc.sync.dma_start(out=prpad[0:1, :], in_=rT[:])

                def chain_tail(item):
                    pc, pj_, pq0, pauSB, prpad = item
                    for c in range(qc // nch):
                        csl = slice(c * nch, (c + 1) * nch)
                        R = p_pool.tile([64, nch], F32, tag="pj", bufs=2, name="Rb")
                        MM(R[:], ones128[:, 0:64], prpad[:, csl],
                           start=True, stop=True)
                        nc.vector.tensor_mul(
                            AN[pc][pj_ * 64:(pj_ + 1) * 64,
                                   pq0 + c * nch:pq0 + (c + 1) * nch],
                            pauSB[0:64, csl], R[:])

                def emit_sa(g):
                    u, kb = divmod(g, tokt)
                    p, j, q = units[u]
                    h = 2 * p + j
                    q0 = q * qc
                    st = s_pool.tile([128, qc], F32, tag="st", bufs=2, name="st")
                    for c in range(qc // nch):
                        MM(st[:, c * nch:(c + 1) * nch],
                           KT[h][:, kb * 128:(kb + 1) * 128],
                           QT[p][:, q0 + c * nch:q0 + (c + 1) * nch],
                           start=True, stop=True)
                    pt = ptp.tile([128, qc], BF16, tag="pt", bufs=LEAD + 1, name="pt")
                    nc.scalar.activation(pt[:], st[:], EXP, scale=0.125)
                    pt_t[g] = pt
                    flist = fillers[u]
                    while fidx[u] < len(flist) and flist[fidx[u]][0] <= kb:
                        flist[fidx[u]][1]()
                        fidx[u] += 1

                def emit_pv(g):
                    u, kb = divmod(g, tokt)
                    p, j, q = units[u]
                    h = 2 * p + j
                    if kb == 0:
                        au_t[u] = u_pool.tile([65, qc], F32, tag="au", bufs=1, name="au")
                    au = au_t[u]
                    pt = pt_t.pop(g)
                    for c in range(qc // nch):
                        MM(au[:, c * nch:(c + 1) * nch],
                           V6[kb][:, h * (HD + 1):(h + 1) * (HD + 1)],
                           pt[:, c * nch:(c + 1) * nch],
                           start=(kb == 0), stop=(kb == tokt - 1))
                    if kb == tokt - 1:
                        auSB = wk.tile([65, qc], F32, tag="ausb", bufs=3, name="ausb")
                        nc.vector.tensor_copy(auSB[:], au[:])
                        del au_t[u]
                        pend.append([2, (p, j, q * qc, auSB, r_pads[u % 2])])

                def tick_chains():
                    for it in pend:
                        it[0] -= 1
                    while pend and pend[0][0] <= 0:
                        it = pend.pop(0)[1]
                        chain_head(it)
                        chain_tail(it)

                for g in range(GS):
                    emit_sa(g)
                    if g >= LEAD:
                        emit_pv(g - LEAD)
                        tick_chains()
                for g in range(GS - LEAD, GS):
                    emit_pv(g)
                # final unit's chain: start the reciprocal round-trip, keep the
                # PE clock warm with dummies while it lands, then finish
                last = pend.pop(0)[1]
                chain_head(last)
                for _ in range(36):
                    MM(wdum[:], ones128[:], r_pads[0][:, 0:nch], start=True, stop=True)
                chain_tail(last)
                # trailing out-proj for the last q-chunk (tokens 1024:2048)
                for c in range(8, 16):
                    oproj_tile(c)
        lp.__exit__(None, None, None)

    return nc


def shard_inputs(x, w_qkv, b_qkv, w_out, b_out, t=T):
    """Build the 8 per-core input maps. Core = (batch, head-group)."""
    dk = D // 128
    nch = 512
    ncc = t // nch
    in_maps = []
    for core in range(NCORES):
        b, g = divmod(core, 2)
        hbase = HL * g * HD
        # q cols then k cols, pair-interleaved: M-tile 2p = q of heads (2p,2p+1),
        # M-tile 2p+1 = k of the same heads.
        wqk = np.empty((D, 2 * HL * HD), dtype=np.float32)
        bqk = np.empty((2 * HL * HD,), dtype=np.float32)
        for p in range(NPAIR):
            qcols = slice(0 * D + hbase + p * 128, 0 * D + hbase + (p + 1) * 128)
            kcols = slice(1 * D + hbase + p * 128, 1 * D + hbase + (p + 1) * 128)
            wqk[:, (2 * p) * 128:(2 * p + 1) * 128] = w_qkv[:, qcols]
            wqk[:, (2 * p + 1) * 128:(2 * p + 2) * 128] = w_qkv[:, kcols]
            bqk[(2 * p) * 128:(2 * p + 1) * 128] = b_qkv[qcols]
            bqk[(2 * p + 1) * 128:(2 * p + 2) * 128] = b_qkv[kcols]
        nmt = 2 * HL * HD // 128
        bqk_col = np.ascontiguousarray(bqk.reshape(nmt, 128).T)  # [128, nmt]

        vcols = slice(2 * D + hbase, 2 * D + hbase + HL * HD)
        wv = np.ascontiguousarray(w_qkv[:, vcols])               # [768, 384]
        wvb_row = b_qkv[vcols].reshape(1, HL * HD)

        xT = np.ascontiguousarray(x[b, :t].T)                    # [768, 2048]
        # xt: [k, p, c, tok] -> [p, c, k, tok]
        xtA = xT.reshape(dk, 128, ncc, nch).transpose(1, 2, 0, 3).reshape(
            128, ncc * dk * nch)
        # weights k-banded: [k, p, col] -> [p, k, col]
        wqk0A = wqk[:, 0:256].reshape(dk, 128, 256).transpose(1, 0, 2).reshape(
            128, dk * 256)
        wqkrA = wqk[:, 256:768].reshape(dk, 128, 512).transpose(1, 0, 2).reshape(
            128, dk * 512)
        wvA = wv.reshape(dk, 128, HL * HD).transpose(1, 0, 2).reshape(
            128, dk * HL * HD)

        wo = np.ascontiguousarray(w_out[hbase:hbase + HL * HD, :])

        in_maps.append(
            {
                "xt": np.ascontiguousarray(xtA).astype(BF),
                "wqk0": np.ascontiguousarray(wqk0A).astype(BF),
                "wqkr": np.ascontiguousarray(wqkrA).astype(BF),
                "bqk": bqk_col.astype(np.float32),
                "wv": np.ascontiguousarray(wvA).astype(BF),
                "wvb": wvb_row.astype(BF),
                "wo": wo.astype(BF),
            }
        )
    return in_maps


def kernel(x, w_qkv, b_qkv, w_out, b_out):
    x = np.asarray(x, dtype=np.float32)
    w_qkv = np.asarray(w_qkv, dtype=np.float32)
    b_qkv = np.asarray(b_qkv, dtype=np.float32)
    w_out = np.asarray(w_out, dtype=np.float32)
    b_out = np.asarray(b_out, dtype=np.float32)

    nc = build_nc()
    _split_multi_waits(nc)
    in_maps = shard_inputs(x, w_qkv, b_qkv, w_out, b_out)
    res = run_bass_kernel_spmd(nc, in_maps, list(range(NCORES)))
    parts = [np.asarray(res.results[i]["out"]).astype(np.float32) for i in range(NCORES)]
    out = np.stack([parts[2 * b] + parts[2 * b + 1] for b in range(B)], axis=0)
    out += b_out[None, None, :]
    return out.astype(np.float32)


# revision 32
# speedup vs baseline: 1.0277x; 1.0152x over previous
"""Trainium2 Bass kernel: multi-head attention (B=4, T=2048, D=768, H=12).

Sharding: 8 cores = 4 batches x 2 head-groups (6 heads each).
Each core computes QKV projection (its heads), attention, and a partial
output projection (contraction over its 384 of 768 w_out rows).
Host unshard: out[b] = f32(partial[2b]) + f32(partial[2b+1]) + b_out.

Fully pipelined single-stream schedule. The ScalarE exp stream (192
tiles of [128,1024], ~213us busy) is the kernel floor; everything else
is arranged around keeping it gap-free:

  - bf16 inputs, DMA'd in consumption order. x^T is laid out in DRAM by
    token-chunk (all six D-tiles of a 512-token chunk contiguous), so
    the first QK projection starts after ~1.6MB, not 4.2MB. q-chunk 0
    (tokens 0:1024) is processed first for the same reason.
  - phase 2 is one flat stream over (unit, kb) steps; S/exp of step g
    and the PV of step g-LEAD are emitted together, crossing unit
    boundaries so the PE never drains PV work while ACT waits for S.
  - all remaining projection and out-projection work is emitted as
    fillers inside designated units (after that step's S matmuls), at a
    density the PE can absorb without starving ACT.
  - softmax denominators ride the PV matmul as a 65th lhsT column of
    ones; the normalize chain per unit is: DVE evict of au, DMA-reshape
    of the denominator row to [128, qc/128], DVE reciprocal (all lanes),
    DMA back into a zero-padded r_pad row, rank-1 PE matmul broadcast
    into 1-bank pj PSUM slots, DVE multiply into AN. The chain is
    deferred two PV-steps so none of it gates the S stream.
  - dummy matmuls warm the PE HAM clock gate during the DMA head and
    the final reciprocal round-trip.
  - PSUM budget (8 banks): st 2x[128,1024] (4) + au [65,1024] (2) +
    shared 1-bank pj pool x2 (2) used by V-proj/QK-proj/out-proj
    fillers, warmup dummies, and the normalize broadcasts.

This walrus build encodes at most one sync wait per instruction; Tile
emits several. _split_multi_waits() hoists extras onto same-engine nops.
"""

import numpy as np
import ml_dtypes

import concourse.bass as bass
import concourse.mybir as mybir
from concourse.tile import TileContext
from concourse.bass_utils import run_bass_kernel_spmd

# problem constants (fixed by the graded nn.Module)
B, T, D = 4, 2048, 768
H, HD = 12, 64
NCORES = 8
HL = H // 2            # heads per core
NPAIR = HL // 2        # head pairs per core

F32 = mybir.dt.float32
BF16 = mybir.dt.bfloat16
BF = ml_dtypes.bfloat16
COPY = mybir.ActivationFunctionType.Copy
IDENT = mybir.ActivationFunctionType.Identity


def _split_multi_waits(nc):
    """Walrus here encodes only one sync wait per instruction. Move extra
    waits onto same-engine nops placed immediately before the instruction."""
    n = 0
    for f in nc.m.functions:
        for bb in f.blocks:
            new = []
            for inst in bb.instructions:
                si = inst.sync_info
                if si is not None and si.on_wait and len(si.on_wait) > 1:
                    extra = list(si.on_wait[:-1])
                    keep = si.on_wait[-1]
                    del si.on_wait[:]
                    si.on_wait.append(keep)
                    for w in extra:
                        nop = mybir.InstNoOp(name=f"I-wsplit-{n}", ins=[], outs=[])
                        n += 1
                        nop.engine = inst.engine
                        nop.sync_info = mybir.SyncInfo(on_wait=[w], on_update=[])
                        new.append(nop)
                new.append(inst)
            bb.instructions[:] = new
    return n


def build_nc(t=T, qc=1024, nch=512):
    tokt = t // 128            # token tiles (16)
    dk = D // 128              # contraction tiles over D (6)
    nmt = 2 * HL * HD // 128   # QK projection M-tiles (6)
    ncc = t // nch             # token chunks (4)
    EXP = mybir.ActivationFunctionType.Exp

    nc = bass.Bass("TRN2", target_bir_lowering=False, debug=False)

    # x^T by token-chunk: col = c*(dk*nch) + k*nch + tok
    xt_d = nc.dram_tensor("xt", [128, ncc * dk * nch], BF16, kind="ExternalInput")
    # weights k-banded: col = k*width + in-tile col
    wqk0_d = nc.dram_tensor("wqk0", [128, dk * 256], BF16, kind="ExternalInput")
    wqkr_d = nc.dram_tensor("wqkr", [128, dk * 512], BF16, kind="ExternalInput")
    wv_d = nc.dram_tensor("wv", [128, dk * HL * HD], BF16, kind="ExternalInput")
    wvb_d = nc.dram_tensor("wvb", [1, HL * HD], BF16, kind="ExternalInput")
    bqk_d = nc.dram_tensor("bqk", [128, nmt], F32, kind="ExternalInput")
    wo_d = nc.dram_tensor("wo", [HL * HD, D], BF16, kind="ExternalInput")
    out_d = nc.dram_tensor("out", [t, D], BF16, kind="ExternalOutput")

    def MM(out, lhsT, rhs, start, stop):
        nc.tensor.matmul(out, lhsT, rhs, start=start, stop=stop)

    with TileContext(nc) as tc:
        lp = nc.allow_low_precision(reason="bf16 compute pipeline")
        lp.__enter__()
        with tc.tile_pool(name="persist", bufs=1) as pp:
            ones128 = pp.tile([128, 128], BF16, name="ones128")
            r_pads = [pp.tile([128, qc], BF16, name=f"r_pad{i}") for i in range(2)]
            QT = [pp.tile([128, t], BF16, name=f"qt{p}") for p in range(NPAIR)]
            KT = [pp.tile([128, t], BF16, name=f"kt{h}") for h in range(HL)]
            V6 = [pp.tile([128, HL * (HD + 1)], BF16, name=f"v6_{c}") for c in range(tokt)]
            AN = [pp.tile([128, t], BF16, name=f"an{p}") for p in range(NPAIR)]
            WO = [pp.tile([128, D], BF16, name=f"wop{p}") for p in range(NPAIR)]
            bqk_t = pp.tile([128, nmt], F32, name="bqk_t")
            xt_all = pp.tile([128, dk * t], BF16, name="xt_all")
            wqk0_t = pp.tile([128, dk * 256], BF16, name="wqk0_t")
            wqkr_t = pp.tile([128, dk * 512], BF16, name="wqkr_t")
            wv_t = pp.tile([128, dk * HL * HD], BF16, name="wv_t")
            wvb = pp.tile([128, HL * HD], BF16, name="wvb")

            # constants first (wvb memset must precede its row DMA)
            nc.vector.memset(ones128[:], 0.0)
            nc.vector.memset(ones128[0:1, :], 1.0)
            for i in range(2):
                nc.vector.memset(r_pads[i][:], 0.0)
            nc.vector.memset(wvb[:], 0.0)
            for h in range(HL):
                if h % 2 == 0:
                    nc.vector.memset(KT[h][64:128, :], 0.0)
                else:
                    nc.vector.memset(KT[h][0:64, :], 0.0)

            # DMAs in consumption order; x^T chunks land one by one
            xt3 = xt_all[:].rearrange("p (k tt) -> p k tt", tt=t)
            nc.sync.dma_start(out=bqk_t[:], in_=bqk_d[:, :])
            for c in (0, 1):
                nc.sync.dma_start(
                    out=xt3[:, :, c * nch:(c + 1) * nch],
                    in_=xt_d[:, c * dk * nch:(c + 1) * dk * nch])
            nc.sync.dma_start(out=wqk0_t[:], in_=wqk0_d[:, :])
            nc.sync.dma_start(out=wv_t[:], in_=wv_d[:, :])
            nc.sync.dma_start(out=wvb[0:1, :], in_=wvb_d[:, :])
            for c in (2, 3):
                nc.sync.dma_start(
                    out=xt3[:, :, c * nch:(c + 1) * nch],
                    in_=xt_d[:, c * dk * nch:(c + 1) * dk * nch])
            nc.sync.dma_start(out=wqkr_t[:], in_=wqkr_d[:, :])
            for p in range(NPAIR):
                nc.sync.dma_start(out=WO[p][:], in_=wo_d[p * 128:(p + 1) * 128, :])

            with (
                tc.tile_pool(name="psum_s", bufs=2, space="PSUM") as s_pool,
                tc.tile_pool(name="psum_u", bufs=1, space="PSUM") as u_pool,
                tc.tile_pool(name="psum_p", bufs=2, space="PSUM") as p_pool,
                tc.tile_pool(name="sb_wk", bufs=2) as wk,
                tc.tile_pool(name="sb_pt", bufs=4) as ptp,
                tc.tile_pool(name="sb_so", bufs=3) as sop,
            ):
                def qk_mtile_chunk(m, c, on_act=False):
                    """One [128,512] token-chunk of QK-projection M-tile m, with
                    bias eviction into QT / zero-padded KT."""
                    csl = slice(c * nch, (c + 1) * nch)
                    ps = p_pool.tile([128, nch], F32, tag="pj", bufs=2, name="psqk")
                    for k in range(dk):
                        if m < 2:
                            w = wqk0_t[:, k * 256 + m * 128:k * 256 + (m + 1) * 128]
                        else:
                            w = wqkr_t[:, k * 512 + (m - 2) * 128:k * 512 + (m - 1) * 128]
                        MM(ps[:], w, xt_all[:, k * t + c * nch:k * t + (c + 1) * nch],
                           start=(k == 0), stop=(k == dk - 1))
                    if m % 2 == 0:
                        if on_act:
                            nc.scalar.activation(QT[m // 2][:, csl], ps[:], IDENT,
                                                 bias=bqk_t[:, m:m + 1])
                        else:
                            nc.vector.tensor_scalar_add(
                                QT[m // 2][:, csl], ps[:], bqk_t[:, m:m + 1])
                    else:
                        pr = m // 2
                        if on_act:
                            nc.scalar.activation(KT[2 * pr][0:64, csl], ps[0:64, :],
                                                 IDENT, bias=bqk_t[0:64, m:m + 1])
                            nc.scalar.activation(KT[2 * pr + 1][64:128, csl],
                                                 ps[64:128, :], IDENT,
                                                 bias=bqk_t[64:128, m:m + 1])
                        else:
                            nc.vector.tensor_scalar_add(
                                KT[2 * pr][0:64, csl], ps[0:64, :], bqk_t[0:64, m:m + 1])
                            nc.vector.tensor_scalar_add(
                                KT[2 * pr + 1][64:128, csl], ps[64:128, :],
                                bqk_t[64:128, m:m + 1])

                def v_ctile(c, on_act=False):
                    """V projection for token tile c -> V6[c] (with ones col)."""
                    tsl = slice(c * 128, (c + 1) * 128)
                    psv = p_pool.tile([128, HL * HD], F32, tag="pj", bufs=2, name="psv")
                    for k in range(dk):
                        MM(psv[:], xt_all[:, k * t + c * 128:k * t + (c + 1) * 128],
                           wv_t[:, k * HL * HD:(k + 1) * HL * HD],
                           start=(k == 0), stop=False)
                    MM(psv[:], ones128[:], wvb[:], start=False, stop=True)
                    v3 = V6[c][:].rearrange("p (h c2) -> p h c2", c2=HD + 1)
                    nc.vector.memset(v3[:, :, HD:HD + 1], 1.0)
                    src = psv[:].rearrange("p (h c2) -> p h c2", c2=HD)
                    if on_act:
                        nc.scalar.activation(v3[:, :, 0:HD], src, COPY)
                    else:
                        nc.vector.tensor_copy(v3[:, :, 0:HD], src)

                def oproj_tile(c):
                    """Output projection for token tile c (two 384-col halves,
                    one eviction tile, one output DMA)."""
                    tsl = slice(c * 128, (c + 1) * 128)
                    so = sop.tile([128, D], BF16, tag="so", bufs=3, name="so")
                    for half in range(2):
                        nsl = slice(half * 384, (half + 1) * 384)
                        ps = p_pool.tile([128, 384], F32, tag="pj", bufs=2, name="pso")
                        for p in range(NPAIR):
                            MM(ps[:], AN[p][:, tsl], WO[p][:, nsl],
                               start=(p == 0), stop=(p == NPAIR - 1))
                        nc.vector.tensor_copy(so[:, nsl], ps[:])
                    eng = nc.scalar if c >= 8 and c % 2 == 1 else nc.sync
                    eng.dma_start(out=out_d[tsl, :], in_=so[:])

                # filler schedule (q0-chunk first): unit -> [(kb, closure)...]
                # emitted AFTER that step's S matmuls
                def QKf(m, c):
                    return lambda: qk_mtile_chunk(m, c)

                fillers = {u: [] for u in range(2 * HL)}
                fillers[0] = sorted(
                    [(kb, (lambda cc=kb + 3: v_ctile(cc))) for kb in range(tokt - 3)]
                    + [(1, QKf(1, 2)), (2, QKf(1, 3))],
                    key=lambda kv: kv[0])
                fillers[1] = [(0, QKf(2, 0)), (2, QKf(2, 1)), (4, QKf(3, 0)),
                              (6, QKf(3, 1)), (8, QKf(3, 2)), (10, QKf(3, 3))]
                fillers[2] = [(0, QKf(4, 0)), (2, QKf(4, 1)), (4, QKf(5, 0)),
                              (6, QKf(5, 1)), (8, QKf(5, 2)), (10, QKf(5, 3))]
                fillers[3] = [(4, QKf(0, 2)), (10, QKf(0, 3))]
                fillers[4] = [(4, QKf(2, 2)), (10, QKf(2, 3))]
                fillers[5] = [(4, QKf(4, 2)), (10, QKf(4, 3))]
                for i, c in enumerate(range(0, 8)):
                    fillers[6 + i // 2].append(
                        (5 + 8 * (i % 2), (lambda cc=c: oproj_tile(cc))))

                # dummy matmuls with no DMA deps keep the PE busy during the
                # input-DMA head so the HAM clock gate reaches 2.4GHz early
                wdum = p_pool.tile([128, nch], F32, tag="pj", bufs=2, name="wdum")
                for _ in range(28):
                    MM(wdum[:], ones128[:], r_pads[0][:, 0:nch], start=True, stop=True)

                # QK projection for pair 0, q-chunk 0 (gates the first S) and
                # the first V tiles; evictions ride the still-idle ScalarE
                for m in (0, 1):
                    for c in (0, 1):
                        qk_mtile_chunk(m, c, on_act=True)
                for c in range(3):
                    v_ctile(c, on_act=True)

                # ---- phase 2: one flat stream over (unit, kb) ----
                units = [(p, j, q) for q in (0, 1)
                         for p in range(NPAIR) for j in range(2)]
                LEAD = 3
                GS = len(units) * tokt
                au_t = {}
                pt_t = {}
                fidx = {u: 0 for u in range(len(units))}
                pend = []

                def chain_head(item):
                    pc, pj_, pq0, pauSB, prpad = item
                    dT = wk.tile([128, qc // 128], F32, tag="dt", bufs=2, name="dt")
                    nc.sync.dma_start(out=dT[:], in_=pauSB[64:65, :])
                    rT = wk.tile([128, qc // 128], BF16, tag="rt", bufs=2, name="rt")
                    nc.vector.reciprocal(rT[:], dT[:])
                    nc.sync.dma_start(out=prpad[0:1, :], in_=rT[:])

                def chain_tail(item):
                    pc, pj_, pq0, pauSB, prpad = item
                    for c in range(qc // nch):
                        csl = slice(c * nch, (c + 1) * nch)
                        R = p_pool.tile([64, nch], F32, tag="pj", bufs=2, name="Rb")
                        MM(R[:], ones128[:, 0:64], prpad[:, csl],
                           start=True, stop=True)
                        nc.vector.tensor_mul(
                            AN[pc][pj_ * 64:(pj_ + 1) * 64,
                                   pq0 + c * nch:pq0 + (c + 1) * nch],
                            pauSB[0:64, csl], R[:])

                def emit_sa(g):
                    u, kb = divmod(g, tokt)
                    p, j, q = units[u]
                    h = 2 * p + j
                    q0 = q * qc
                    st = s_pool.tile([128, qc], F32, tag="st", bufs=2, name="st")
                    for c in range(qc // nch):
                        MM(st[:, c * nch:(c + 1) * nch],
                           KT[h][:, kb * 128:(kb + 1) * 128],
                           QT[p][:, q0 + c * nch:q0 + (c + 1) * nch],
                           start=True, stop=True)
                    pt = ptp.tile([128, qc], BF16, tag="pt", bufs=LEAD + 1, name="pt")
                    nc.scalar.activation(pt[:], st[:], EXP, scale=0.125)
                    pt_t[g] = pt
                    flist = fillers[u]
                    while fidx[u] < len(flist) and flist[fidx[u]][0] <= kb:
                        flist[fidx[u]][1]()
                        fidx[u] += 1

                def emit_pv(g):
                    u, kb = divmod(g, tokt)
                    p, j, q = units[u]
                    h = 2 * p + j
                    if kb == 0:
                        au_t[u] = u_pool.tile([65, qc], F32, tag="au", bufs=1, name="au")
                    au = au_t[u]
                    pt = pt_t.pop(g)
                    for c in range(qc // nch):
                        MM(au[:, c * nch:(c + 1) * nch],
                           V6[kb][:, h * (HD + 1):(h + 1) * (HD + 1)],
                           pt[:, c * nch:(c + 1) * nch],
                           start=(kb == 0), stop=(kb == tokt - 1))
                    if kb == tokt - 1:
                        auSB = wk.tile([65, qc], F32, tag="ausb", bufs=3, name="ausb")
                        nc.vector.tensor_copy(auSB[:], au[:])
                        del au_t[u]
                        pend.append([2, (p, j, q * qc, auSB, r_pads[u % 2])])

                def tick_chains():
                    for it in pend:
                        it[0] -= 1
                    while pend and pend[0][0] <= 0:
                        it = pend.pop(0)[1]
                        chain_head(it)
                        chain_tail(it)

                for g in range(GS):
                    emit_sa(g)
                    if g >= LEAD:
                        emit_pv(g - LEAD)
                        tick_chains()
                for g in range(GS - LEAD, GS):
                    emit_pv(g)
                # final unit's chain: start the reciprocal round-trip, keep the
                # PE clock warm with dummies while it lands, then finish
                last = pend.pop(0)[1]
                chain_head(last)
                for _ in range(36):
                    MM(wdum[:], ones128[:], r_pads[0][:, 0:nch], start=True, stop=True)
                chain_tail(last)
                # trailing out-proj for the last q-chunk (tokens 1024:2048)
                for c in range(8, 16):
                    oproj_tile(c)
        lp.__exit__(None, None, None)

    return nc


def shard_inputs(x, w_qkv, b_qkv, w_out, b_out, t=T):
    """Build the 8 per-core input maps. Core = (batch, head-group)."""
    dk = D // 128
    nch = 512
    ncc = t // nch
    in_maps = []
    for core in range(NCORES):
        b, g = divmod(core, 2)
        hbase = HL * g * HD
        # q cols then k cols, pair-interleaved: M-tile 2p = q of heads (2p,2p+1),
        # M-tile 2p+1 = k of the same heads.
        wqk = np.empty((D, 2 * HL * HD), dtype=np.float32)
        bqk = np.empty((2 * HL * HD,), dtype=np.float32)
        for p in range(NPAIR):
            qcols = slice(0 * D + hbase + p * 128, 0 * D + hbase + (p + 1) * 128)
            kcols = slice(1 * D + hbase + p * 128, 1 * D + hbase + (p + 1) * 128)
            wqk[:, (2 * p) * 128:(2 * p + 1) * 128] = w_qkv[:, qcols]
            wqk[:, (2 * p + 1) * 128:(2 * p + 2) * 128] = w_qkv[:, kcols]
            bqk[(2 * p) * 128:(2 * p + 1) * 128] = b_qkv[qcols]
            bqk[(2 * p + 1) * 128:(2 * p + 2) * 128] = b_qkv[kcols]
        nmt = 2 * HL * HD // 128
        bqk_col = np.ascontiguousarray(bqk.reshape(nmt, 128).T)  # [128, nmt]

        vcols = slice(2 * D + hbase, 2 * D + hbase + HL * HD)
        wv = np.ascontiguousarray(w_qkv[:, vcols])               # [768, 384]
        wvb_row = b_qkv[vcols].reshape(1, HL * HD)

        xT = np.ascontiguousarray(x[b, :t].T)                    # [768, 2048]
        # xt: [k, p, c, tok] -> [p, c, k, tok]
        xtA = xT.reshape(dk, 128, ncc, nch).transpose(1, 2, 0, 3).reshape(
            128, ncc * dk * nch)
        # weights k-banded: [k, p, col] -> [p, k, col]
        wqk0A = wqk[:, 0:256].reshape(dk, 128, 256).transpose(1, 0, 2).reshape(
            128, dk * 256)
        wqkrA = wqk[:, 256:768].reshape(dk, 128, 512).transpose(1, 0, 2).reshape(
            128, dk * 512)
        wvA = wv.reshape(dk, 128, HL * HD).transpose(1, 0, 2).reshape(
            128, dk * HL * HD)

        wo = np.ascontiguousarray(w_out[hbase:hbase + HL * HD, :])

        in_maps.append(
            {
                "xt": np.ascontiguousarray(xtA).astype(BF),
                "wqk0": np.ascontiguousarray(wqk0A).astype(BF),
                "wqkr": np.ascontiguousarray(wqkrA).astype(BF),
                "bqk": bqk_col.astype(np.float32),
                "wv": np.ascontiguousarray(wvA).astype(BF),
                "wvb": wvb_row.astype(BF),
                "wo": wo.astype(BF),
            }
        )
    return in_maps


def kernel(x, w_qkv, b_qkv, w_out, b_out):
    x = np.asarray(x, dtype=np.float32)
    w_qkv = np.asarray(w_qkv, dtype=np.float32)
    b_qkv = np.asarray(b_qkv, dtype=np.float32)
    w_out = np.asarray(w_out, dtype=np.float32)
    b_out = np.asarray(b_out, dtype=np.float32)

    nc = build_nc()
    _split_multi_waits(nc)
    in_maps = shard_inputs(x, w_qkv, b_qkv, w_out, b_out)
    res = run_bass_kernel_spmd(nc, in_maps, list(range(NCORES)))
    parts = [np.asarray(res.results[i]["out"]).astype(np.float32) for i in range(NCORES)]
    out = np.stack([parts[2 * b] + parts[2 * b + 1] for b in range(B)], axis=0)
    out += b_out[None, None, :]
    return out.astype(np.float32)
